# revision 24
# baseline (speedup 1.0000x reference)
"""Causal single-head attention  B=4, T=4096, C=1024, D=64  on 8 TRN2 cores.

Sharding: 2 cores per batch; core parity p takes query rows {2i+p}.
Even/odd interleave balances causal work exactly across the pair.

v8 (v5 + diagonal-first tile order):
  - global S/P pair pipeline: scores+exp ("S") runs ahead of probs@V
    ("P") by LAG pairs, crossing tile boundaries.
  - diagonal pairs run FIRST within each tile (kv{t} is pumped during
    tile t-1), so the S-stream ends on plain off-diagonal pairs and the
    Activation engine never overhangs the tail; the final PSUM
    evacuation is split across DVE and ACT.
  - k/v psum evacuated with ONE (128,512) copy into a combined kvt tile
    (k rows 0:64, v rows 64:128); the v-transpose identity sits at
    partitions 64:127 (shipped in the masks input) so the PE transposes
    read v in place.
  - x loaded as 16 half-block strided DMAs (3D dram layout) up front on
    the SP queue; PE warmup chain burns the p-state ramp; one 1024-col
    exp per off-diagonal pair; 128-col band masks; vaug ones on Pool.
  - bf16 matmuls, unstabilized exp (scale 0.125), out row 64 =
    denominator, host divides.
"""

import sys

sys.path.insert(0, "/opt/trn_rl_repo")

import numpy as np
import ml_dtypes

B, T, C, D = 4, 4096, 1024, 64
TQ = T // 2            # local queries per core
NT = 4                 # q tiles of 512
QF = 512               # q free-dim tile
KC = 128               # kv chunk
NCC = C // 128         # 8 contraction chunks
N_CORES = 8
WARMUP = 6             # PE p-state warmup matmuls
LAG = 8                # P-stream trails S-stream by this many pairs

_compiled = None


def _build_nc(loop_n=None):
    import contextlib
    import concourse.bass as bass
    import concourse.bacc as bacc
    import concourse.mybir as mybir
    from concourse.tile import TileContext

    f32 = mybir.dt.float32
    bf16 = mybir.dt.bfloat16

    nc = bacc.Bacc("TRN2", target_bir_lowering=False, debug=False)
    xt = nc.dram_tensor("xt", (128, NCC, T), bf16, kind="ExternalInput")
    wq = nc.dram_tensor("wq", (128, NCC * D), bf16, kind="ExternalInput")
    wkv = nc.dram_tensor("wkv", (128, NCC * 2 * D), bf16, kind="ExternalInput")
    masks = nc.dram_tensor("masks", (KC, 2 * KC + 64), bf16, kind="ExternalInput")
    out = nc.dram_tensor("out", (D + 1, TQ), f32, kind="ExternalOutput")

    with TileContext(nc) as tc:
        with (
            tc.tile_pool(name="const", bufs=1) as constp,
            tc.tile_pool(name="xtp", bufs=16) as xtp,
            tc.tile_pool(name="kvtp", bufs=10) as kvtp,
            tc.tile_pool(name="qtp", bufs=8) as qtp,
            tc.tile_pool(name="vaug", bufs=64) as vaugp,
            tc.tile_pool(name="probs", bufs=12) as probsp,
            tc.tile_pool(name="osb", bufs=2) as osbp,
            tc.tile_pool(name="ps_kv", bufs=1, space="PSUM") as ps_kvp,
            tc.tile_pool(name="ps_scr", bufs=2, space="PSUM") as ps_scrp,
            tc.tile_pool(name="ps_s", bufs=2, space="PSUM") as ps_sp,
            tc.tile_pool(name="ps_o", bufs=1, space="PSUM") as ps_op,
        ):
            wq_sb = constp.tile([128, NCC * D], bf16, tag="wq")
            wkv_sb = constp.tile([128, NCC * 2 * D], bf16, tag="wkv")
            mask_sb = constp.tile([KC, 2 * KC + 64], bf16, tag="masks")
            warm_sb = constp.tile([128, QF], bf16, tag="warm")

            loop_cm = (
                tc.For_i(0, loop_n, 1) if loop_n else contextlib.nullcontext()
            )
            with loop_cm:
              # ---- PE warmup: engine busy from ~t=0 so the p-state ramp
              # (full speed only after 3us of continuous execution) is burnt
              # on dummy work while the first DMAs stream in.
              nc.vector.memset(warm_sb, 0.01)
              ps_w = ps_scrp.tile([128, QF], f32, tag="scr")
              for _ in range(WARMUP):
                  nc.tensor.matmul(
                      ps_w, lhsT=warm_sb[:, 0:128], rhs=warm_sb,
                      start=True, stop=True, skip_group_check=True,
                  )

              # ---- all input DMAs up front on the SP queue, in consumption
              # order. xh[(b, h)] = chunks 4h..4h+3 of 512-col block b
              # (blocks 0-3 own parity, 4-7 other).
              xh = {}

              def xload(b, h):
                  xtile = xtp.tile([128, 4, QF], bf16, tag="xt")
                  nc.sync.dma_start(
                      out=xtile,
                      in_=xt[:, 4 * h:4 * h + 4, b * QF:(b + 1) * QF],
                  )
                  xh[(b, h)] = xtile

              # first quarter-block races in via the otherwise-idle Pool
              # queue so the q0 projection can start ~1us earlier
              x00 = xtp.tile([128, 2, QF], bf16, tag="xt0")
              nc.gpsimd.dma_start(out=x00, in_=xt[:, 0:2, 0:QF])
              x01 = xtp.tile([128, 2, QF], bf16, tag="xt0")
              nc.sync.dma_start(out=wq_sb, in_=wq[:, :])
              nc.sync.dma_start(out=x01, in_=xt[:, 2:4, 0:QF])
              xload(0, 1)
              nc.sync.dma_start(out=wkv_sb, in_=wkv[:, :])
              xload(4, 0)
              xload(4, 1)
              nc.sync.dma_start(out=mask_sb, in_=masks[:, :])
              for bp in (1, 2, 3):
                  for b in (bp, bp + 4):
                      xload(b, 0)
                      xload(b, 1)

              # identity at partitions 64:127 (cols 256:320 of masks) for
              # the in-place v transposes
              identq = mask_sb[64:128, 2 * KC:2 * KC + 64]

              def xch(b, c):
                  if b == 0 and c < 4:
                      return (x00, x01)[c // 2][:, c % 2, :]
                  return xh[(b, c // 4)][:, c % 4, :]

              kts = {}     # block b (0-3 own, 4-7 other) -> (64, 512) bf16 kT
              qts = {}     # q tile t -> (64, 512) bf16
              vaug = {}    # kv chunk id (s*16 + 4*bp + j) -> (128, 65) bf16

              def gq(bp):
                  # q projection of tile bp; must finish before attn(bp)
                  ps_q = ps_scrp.tile([64, QF], f32, tag="scr")
                  for c0 in range(0, NCC, 2):
                      for c in (c0, c0 + 1):
                          nc.tensor.matmul(
                              ps_q,
                              lhsT=wq_sb[:, c * D:(c + 1) * D],
                              rhs=xch(bp, c),
                              start=(c == 0),
                              stop=(c == NCC - 1),
                              skip_group_check=True,
                          )
                      yield 1
                  qt_t = qtp.tile([64, QF], bf16, tag="qt")
                  nc.vector.tensor_copy(qt_t, ps_q)
                  qts[bp] = qt_t
                  yield 1

              def gkv(bp):
                  # k/v projections of both parities; must finish before
                  # attn(bp) starts (its diagonal pairs come first).
                  for s in (0, 1):
                      b = bp + 4 * s
                      if bp == 0 and s == 1:
                          # the score-psum pool is still unused this early:
                          # borrow a tile so s1 doesn't wait for s0's copy
                          ps_kv = ps_sp.tile(
                              [128, 2, QF], f32, tag="pss", name="ps_kv1"
                          )[:, 0, :]
                      else:
                          ps_kv = ps_kvp.tile([128, QF], f32, tag="pskv")
                      for c0 in range(0, NCC, 2):
                          for c in (c0, c0 + 1):
                              nc.tensor.matmul(
                                  ps_kv,
                                  lhsT=wkv_sb[:, c * 2 * D:(c + 1) * 2 * D],
                                  rhs=xch(b, c),
                                  start=(c == 0),
                                  stop=(c == NCC - 1),
                                  skip_group_check=True,
                              )
                          yield 1
                      kvt = kvtp.tile([128, QF], bf16, tag="kvt")
                      nc.vector.tensor_copy(kvt, ps_kv)
                      kts[b] = kvt[0:64, :]
                      yield 1
                      for j in range(4):
                          ps_v = ps_scrp.tile([128, 64], bf16, tag="scr")
                          nc.tensor.transpose(
                              ps_v, kvt[64:128, j * 128:(j + 1) * 128], identq
                          )
                          va = vaugp.tile([128, D + 1], bf16, tag="vaug")
                          nc.vector.tensor_copy(va[:, 0:D], ps_v)
                          nc.gpsimd.memset(va[:, D:D + 1], 1.0)
                          vaug[s * 16 + 4 * bp + j] = va
                          if j == 1 or j == 3:
                              yield 1

              import itertools

              class Pump:
                  # segs = [(name, gen, n_units)]; marks are cumulative so
                  # attention can spread units evenly toward each barrier
                  def __init__(self, segs):
                      self.marks = {}
                      c = 0
                      gens = []
                      for n, g, sz in segs:
                          c += sz
                          self.marks[n] = c
                          gens.append(g)
                      self.stream = itertools.chain(*gens)
                      self.count = 0
                      self.total = c

                  def pump(self, k):
                      for _ in range(k):
                          if next(self.stream, None) is None:
                              self.count = self.total
                              return
                          self.count += 1

                  def pump_to(self, tgt):
                      while self.count < tgt:
                          if next(self.stream, None) is None:
                              self.count = self.total
                              return
                          self.count += 1

              # ---- global pair stream across all tiles, diagonals first.
              # pair = (tile, kind, s, block bp, base chunk j0)
              pairs = []
              first_step = {}
              first_diag = {}
              for t in range(NT):
                  first_step[t] = len(pairs)
                  for bp in range(t):
                      for s in (0, 1):
                          for c in (0, 1):
                              pairs.append((t, "off", s, bp, 2 * c))
                  first_diag[t] = len(pairs)
                  for s in (0, 1):
                      for rr in (0, 2):
                          pairs.append((t, "diag", s, t, rr))
              NP = len(pairs)
              last_step = {t: first_step.get(t + 1, NP) - 1 for t in range(NT)}

              probs = {}
              ps_o_cur = [None]

              def emitS(g):
                  t, kind, s, bp, j0 = pairs[g]
                  qt = qts[t]
                  kt_t = kts[bp + 4 * s]
                  ps2 = ps_sp.tile([128, 2, QF], f32, tag="pss")
                  for u in (0, 1):
                      j = j0 + u
                      c0 = 128 * j if kind == "diag" else 0
                      nc.tensor.matmul(
                          ps2[:, u, c0:],
                          lhsT=kt_t[:, j * 128:(j + 1) * 128],
                          rhs=qt[:, c0:],
                          start=True,
                          stop=True,
                          skip_group_check=True,
                      )
                  p2 = probsp.tile([128, 2, QF], bf16, tag="p")
                  if kind == "off":
                      # one 1024-col exp covers the chunk pair
                      nc.scalar.activation(
                          p2, ps2,
                          mybir.ActivationFunctionType.Exp, scale=0.125,
                      )
                  else:
                      for u in (0, 1):
                          c0 = 128 * (j0 + u)
                          nc.scalar.activation(
                              p2[:, u, c0:], ps2[:, u, c0:],
                              mybir.ActivationFunctionType.Exp, scale=0.125,
                          )
                          # causal band mask: only cols [c0, c0+128) are
                          # partial; beyond them the mask is all-ones.
                          nc.vector.tensor_mul(
                              p2[:, u, c0:c0 + 128],
                              p2[:, u, c0:c0 + 128],
                              mask_sb[:, s * KC:(s + 1) * KC],
                          )
                  probs[g] = p2

              def emitP(g):
                  t, kind, s, bp, j0 = pairs[g]
                  p2 = probs.pop(g)
                  if g == first_step[t]:
                      ps_o_cur[0] = ps_op.tile(
                          [D + 1, QF], f32, tag="pso", name="ps_o"
                      )
                  ps_o = ps_o_cur[0]
                  for u in (0, 1):
                      j = j0 + u
                      c0 = 128 * j if kind == "diag" else 0
                      nc.tensor.matmul(
                          ps_o[:, c0:],
                          lhsT=vaug[s * 16 + 4 * bp + j],
                          rhs=p2[:, u, c0:],
                          start=(g == first_step[t] and u == 0),
                          stop=(g == last_step[t] and u == 1),
                          skip_group_check=True,
                      )
                  if g == last_step[t]:
                      o_sb = osbp.tile([D + 1, QF], f32, tag="osb")
                      nc.vector.tensor_copy(o_sb, ps_o)
                      nc.sync.dma_start(
                          out=out[:, t * QF:(t + 1) * QF], in_=o_sb
                      )

              # q/kv of pair 0 up front; later pairs pumped through the
              # S-stream.
              for _ in gq(0):
                  pass
              segs = [("kv0", gkv(0), 14)]
              for bp in range(1, 4):
                  segs += [(f"q{bp}", gq(bp), 5), (f"kv{bp}", gkv(bp), 14)]
              pump = Pump(segs)

              # barriers: tile t's off-pairs need q{t}; its diagonal pairs
              # (which come last) need kv{t}
              barriers = []
              for t in range(NT):
                  if t > 0:
                      barriers.append((first_step[t], pump.marks[f"q{t}"]))
                  barriers.append((first_diag[t], pump.marks[f"kv{t}"]))
              barriers.append((NP, pump.total))

              for step in range(NP + LAG):
                  g = step
                  if g < NP:
                      while barriers and barriers[0][0] <= g:
                          pump.pump_to(barriers[0][1])
                          barriers.pop(0)
                      emitS(g)
                      nb_step, nb_mark = barriers[0]
                      need = nb_mark - pump.count
                      slots = nb_step - g
                      if slots > 0 and need > 0:
                          pump.pump(-(-need // slots))
                  if step >= LAG:
                      emitP(step - LAG)
              pump.pump_to(pump.total)

    nc.compile()
    return nc


def _get_compiled():
    global _compiled
    if _compiled is None:
        _compiled = _build_nc()
    return _compiled


def _host_inputs(x, Wq, Wk, Wv):
    bf = ml_dtypes.bfloat16
    # pack (1024, d) weights chunk-wise along columns: (128, 8*d)
    wq = np.concatenate(
        [Wq[c * 128:(c + 1) * 128] for c in range(C // 128)], axis=1
    ).astype(bf)
    wkv_full = np.concatenate([Wk, Wv], axis=1)
    wkv = np.concatenate(
        [wkv_full[c * 128:(c + 1) * 128] for c in range(C // 128)], axis=1
    ).astype(bf)

    j = np.arange(KC)[:, None]   # kv row within chunk
    f = np.arange(KC)[None, :]   # q col within band
    eye = np.zeros((KC, 64), dtype=np.float32)
    eye[64:128] = np.eye(64)
    in_maps = []
    for core in range(N_CORES):
        b, p = core // 2, core % 2
        xs = x[b, p::2]          # (2048, 1024) same parity
        xo = x[b, 1 - p::2]
        xkvT = np.concatenate([xs, xo], axis=0).T    # (1024, 4096)
        x3d = np.ascontiguousarray(
            xkvT.reshape(NCC, 128, T).transpose(1, 0, 2), dtype=bf
        )                        # (128, 8, 4096)
        m_own = (j <= f).astype(np.float32)
        m_oth = (j <= f - (1 - p)).astype(np.float32)
        mask = np.concatenate([m_own, m_oth, eye], axis=1).astype(bf)
        in_maps.append({"xt": x3d, "wq": wq, "wkv": wkv, "masks": mask})
    return in_maps


def kernel(x, Wq, Wk, Wv):
    from concourse.bass_utils import run_bass_kernel_spmd

    nc = _get_compiled()
    in_maps = _host_inputs(x, Wq, Wk, Wv)
    res = run_bass_kernel_spmd(nc, in_maps, core_ids=list(range(N_CORES)))

    out_full = np.empty((B, T, D), dtype=np.float32)
    for core in range(N_CORES):
        b, p = core // 2, core % 2
        acc = res.results[core]["out"]          # (65, 2048) f32
        out_full[b, p::2, :] = (acc[:D] / acc[D:D + 1]).T
    return out_full
